# revision 6
# baseline (speedup 1.0000x reference)
"""Contrastive-loss kernel for Trainium2 (8 NeuronCores, Bass/Tile).

Math (reference):
    W = wsi[:, 0, :], O = omic[:, 0, :]                      # [N, D]
    S = (W @ O.T) / max(|W_i||O_j|, eps)                     # [N, N] cosine sims
    d = diag(S)
    L = where(eye, 1 - S, relu(M - S + d[:, None]))
    out = mean(L)

Device identity:
    sum(L) = sum_{i != j} relu(hb_i - S_ij) + sum_i (1 - d_i),  hb_i = M + d_i.
    The d_i are computed exactly on the host (f64), so the device only has to
    produce sum_{i != j} relu(hb_i - S_ij). The diagonal is excluded on device
    by adding +256 to S_ii inside the PSUM accumulation group (one extra tiny
    matmul against a DoubleRow-packed scaled identity), which clamps the
    diagonal hinge to exactly 0 for both hinge formulations below.

Distribution: data-parallel over W rows. Each core c gets its 512 W rows
(pre-normalized, fp8-e4m3, DoubleRow-packed) plus the full normalized O,
column-rotated by 512*c so the diagonal block always lands in j-chunk 0
(keeps the SPMD program core-independent). Each core computes its
[512, 4096] block of S on the PE (fp8 DoubleRow, fp32 psum); the hinge and
its row-sum are fused into ONE instruction per block with accum_out,
alternating between the Scalar engine (relu(hb - S), sign +1) and the
Vector engine (min(S - hb, 0), sign -1) so neither engine becomes the
straggler. One ones-matmul collapses partitions so the output DMA is a
single 128-byte partition line.
"""

import numpy as np
import ml_dtypes

N = 4096
D = 1024
NCORES = 8
ROWS = N // NCORES  # 512 W rows per core
P = 128             # SBUF partitions
NJ = 512            # moving free dim per matmul (one PSUM bank of fp32)
TI = ROWS // P      # 4 i-tiles per core
ND2 = D // 256      # 4 DoubleRow contraction chunks (256 deep each)
NJC = N // NJ       # 8 j-chunks
MARGIN = 0.1
N_WARMUP = 4        # wide bf16 PE-warmup matmuls issued while DMAs stream
NBLK = NJC * TI     # 32 hinge blocks; even idx -> Scalar, odd idx -> Vector
NHALF = NBLK // 2   # 16 acc columns per engine
POISON = 256.0      # added to S_ii on device (16 * 16 in fp8, exact)

_cache = {}


def _build():
    from contextlib import ExitStack
    import concourse.bacc as bacc
    import concourse.tile as tile
    import concourse.mybir as mybir

    f32 = mybir.dt.float32
    bf16 = mybir.dt.bfloat16
    fp8 = mybir.dt.float8e4

    nc = bacc.Bacc("TRN2", target_bir_lowering=False, debug=False,
                   num_devices=NCORES)
    wt_d = nc.dram_tensor("wt", [P, TI * ND2, 2, P], fp8,
                          kind="ExternalInput").ap()
    ot_d = nc.dram_tensor("ot", [P, NJC * ND2, 2, NJ], fp8,
                          kind="ExternalInput").ap()
    pid_d = nc.dram_tensor("pid", [P, 2, P], fp8, kind="ExternalInput").ap()
    hb_d = nc.dram_tensor("hb", [P, TI], f32, kind="ExternalInput").ap()
    out_d = nc.dram_tensor("out", [1, NBLK], f32, kind="ExternalOutput").ap()

    with tile.TileContext(nc) as tc, ExitStack() as ctx:
        const = ctx.enter_context(tc.tile_pool(name="const", bufs=1))
        otp = ctx.enter_context(tc.tile_pool(name="otp", bufs=NJC))
        pp = ctx.enter_context(tc.tile_pool(name="pp", bufs=6, space="PSUM"))
        pp1 = ctx.enter_context(tc.tile_pool(name="pp1", bufs=1, space="PSUM"))
        scrs = ctx.enter_context(tc.tile_pool(name="scrs", bufs=2))
        scrv = ctx.enter_context(tc.tile_pool(name="scrv", bufs=2))
        smallp = ctx.enter_context(tc.tile_pool(name="small", bufs=1))

        # DMA order puts the first matmul's operands (t=0 weights + j-chunk 0)
        # at the head of the HWDGE queue; hb/pid follow (needed by the first
        # hinge ~0.9us later) and the rest streams behind.
        wt_sb = const.tile([P, TI * ND2, 2, P], fp8, tag="wt")
        nc.sync.dma_start(out=wt_sb[:, 0:ND2, :, :], in_=wt_d[:, 0:ND2, :, :])
        ots = []
        o0 = otp.tile([P, ND2, 2, NJ], fp8, tag="ot")
        nc.sync.dma_start(out=o0[:], in_=ot_d[:, 0:ND2, :, :])
        ots.append(o0)
        hb = const.tile([P, TI], f32, tag="hb")
        nc.sync.dma_start(out=hb[:], in_=hb_d[:, :])
        pid_sb = const.tile([P, 2, P], fp8, tag="pid")
        nc.sync.dma_start(out=pid_sb[:], in_=pid_d[:, :])
        nc.sync.dma_start(out=wt_sb[:, ND2:, :, :], in_=wt_d[:, ND2:, :, :])
        for jc in range(1, NJC):
            o = otp.tile([P, ND2, 2, NJ], fp8, tag="ot")
            nc.sync.dma_start(out=o[:],
                              in_=ot_d[:, jc * ND2:(jc + 1) * ND2, :, :])
            ots.append(o)
        ones_sb = const.tile([P, 1], f32, tag="ones")
        nc.vector.memset(ones_sb[:], 1.0)

        # Warm the PE clock (HAM gate releases after ~3.4us of sustained
        # array activity): wide 512-col matmuls fill the window between
        # program start and the first operand DMA landing, so the HAM
        # release timer starts ~1.5us earlier than the real stream.
        warm_w = const.tile([P, P], bf16, tag="warmw")
        nc.vector.memset(warm_w[:], 0.0)
        warm_rhs = const.tile([P, NJ], bf16, tag="warmrhs")
        nc.vector.memset(warm_rhs[:], 0.0)
        warm_ps = pp1.tile([P, NJ], f32, tag="warmps")
        for _ in range(N_WARMUP):
            nc.tensor.matmul(warm_ps[:], lhsT=warm_w[:], rhs=warm_rhs[:],
                             start=True, stop=True)

        # per-block fused hinge row-sums, one acc tile per consumer engine
        acc_s = const.tile([P, NHALF], f32, tag="accs")
        acc_v = const.tile([P, NHALF], f32, tag="accv")

        for jc in range(NJC):
            for t in range(TI):
                ps = pp.tile([P, NJ], f32, tag="ps")
                for dd in range(ND2):
                    nc.tensor.matmul(
                        ps[:],
                        lhsT=wt_sb[:, t * ND2 + dd, :, :],
                        rhs=ots[jc][:, dd, :, :],
                        start=(dd == 0),
                        stop=(dd == ND2 - 1),
                        perf_mode=mybir.MatmulPerfMode.DoubleRow,
                    )
                    if jc == 0 and dd == 0:
                        # poison the diagonal: S_ii += 256 so the hinge
                        # clamps to 0 there (diag handled analytically on
                        # host). Mid-group partial-range accumulate.
                        nc.tensor.matmul(
                            ps[:, t * P:(t + 1) * P],
                            lhsT=pid_sb[:],
                            rhs=pid_sb[:],
                            start=False,
                            stop=False,
                            perf_mode=mybir.MatmulPerfMode.DoubleRow,
                            skip_group_check=True,
                        )
                idx = jc * TI + t
                col = idx // 2
                if idx % 2 == 0:
                    # Scalar engine: acc_s[col] = sum_j relu(hb - S)
                    h = scrs.tile([P, NJ], bf16, tag="hs")
                    nc.scalar.activation(
                        out=h[:],
                        in_=ps[:],
                        func=mybir.ActivationFunctionType.Relu,
                        bias=hb[:, t:t + 1],
                        scale=-1.0,
                        accum_out=acc_s[:, col:col + 1],
                    )
                else:
                    # Vector engine: with accum_out, op1 is the REDUCTION op:
                    # acc_v[col] = sum_j min(S, hb). Host identity:
                    # sum_j relu(hb - S) = 512*hb - sum_j min(S, hb).
                    h = scrv.tile([P, NJ], bf16, tag="hv")
                    nc.vector.tensor_scalar(
                        out=h[:],
                        in0=ps[:],
                        scalar1=hb[:, t:t + 1],
                        scalar2=None,
                        op0=mybir.AluOpType.min,
                        op1=mybir.AluOpType.add,
                        accum_out=acc_v[:, col:col + 1],
                    )

        # cross-partition reduce on the PE (ones^T @ acc -> [1, 16] each) so
        # the output DMA is one contiguous partition line instead of 128
        # 4-byte descriptors (whose completion receipts dominate the tail)
        tot_ps = pp1.tile([1, NBLK], f32, tag="totps")
        nc.tensor.matmul(tot_ps[:, 0:NHALF], lhsT=ones_sb[:], rhs=acc_s[:, :],
                         start=True, stop=True)
        nc.tensor.matmul(tot_ps[:, NHALF:NBLK], lhsT=ones_sb[:],
                         rhs=acc_v[:, :], start=True, stop=True,
                         skip_group_check=True)
        total = smallp.tile([1, NBLK], f32, tag="tot")
        nc.vector.tensor_copy(total[:], tot_ps[:])
        nc.sync.dma_start(out=out_d[:, :], in_=total[:])

    nc.compile()
    return nc


def _get_nc():
    if "nc" not in _cache:
        _cache["nc"] = _build()
    return _cache["nc"]


def _prep_inputs(wsi, omic):
    fp8np = ml_dtypes.float8_e4m3
    W = np.asarray(wsi, dtype=np.float32)[:, 0, :].astype(np.float64)
    O = np.asarray(omic, dtype=np.float32)[:, 0, :].astype(np.float64)
    Wn = W / np.maximum(np.linalg.norm(W, axis=1, keepdims=True), 1e-30)
    On = O / np.maximum(np.linalg.norm(O, axis=1, keepdims=True), 1e-30)
    d_exact = np.einsum("nd,nd->n", Wn, On)  # exact cos(w_i, o_i)
    hb_all = (MARGIN + d_exact).astype(np.float32)
    Wn8 = Wn.astype(fp8np)
    On8 = On.astype(fp8np)
    # DoubleRow-packed sqrt(POISON)*I: pid[p, 0, m] = 16*delta(p, m)
    pid = np.zeros((P, 2, P), dtype=fp8np)
    pid[np.arange(P), 0, np.arange(P)] = fp8np(16.0)

    in_maps = []
    for c in range(NCORES):
        Wc = Wn8[c * ROWS:(c + 1) * ROWS]  # [512, 1024]
        # wt[p, t*ND2+dd, r, m] = Wc[t*128+m, dd*256 + r*128 + p]
        wt = np.ascontiguousarray(
            Wc.reshape(TI, P, ND2, 2, P).transpose(4, 0, 2, 3, 1)
            .reshape(P, TI * ND2, 2, P))
        # column rotation: permuted col j' <-> original O row (j' + 512c) % N
        Operm = np.roll(On8, -ROWS * c, axis=0)
        # ot[p, jc*ND2+dd, r, n] = Operm[jc*512 + n, dd*256 + r*128 + p]
        ot = np.ascontiguousarray(
            Operm.reshape(NJC, NJ, ND2, 2, P).transpose(4, 0, 2, 3, 1)
            .reshape(P, NJC * ND2, 2, NJ))
        # hb[p, t] = MARGIN + d_exact[c*512 + t*128 + p]
        hbc = np.ascontiguousarray(
            hb_all[c * ROWS:(c + 1) * ROWS].reshape(TI, P).T)
        in_maps.append({"wt": wt, "ot": ot, "pid": pid, "hb": hbc})
    return in_maps, d_exact


def kernel(wsi_embeddings, omic_embeddings):
    from concourse.bass_utils import run_bass_kernel_spmd

    nc = _get_nc()
    in_maps, d_exact = _prep_inputs(wsi_embeddings, omic_embeddings)
    res = run_bass_kernel_spmd(nc, in_maps, list(range(NCORES)))
    # device columns: out[0:16] = Scalar sums of relu(hb - S) for even
    # blocks (t in {0, 2}); out[16:32] = Vector sums of min(S, hb) for odd
    # blocks (t in {1, 3}), corrected on host via
    #   sum_j relu(hb - S) = 512*hb_i - sum_j min(S_ij, hb_i).
    # Host adds the analytic diagonal term sum_i (1 - d_i).
    hb64 = (MARGIN + d_exact).astype(np.float32).astype(np.float64)
    grand = float(np.sum(1.0 - d_exact))
    for c in range(NCORES):
        o = res.results[c]["out"].astype(np.float64).ravel()
        hbc = hb64[c * ROWS:(c + 1) * ROWS].reshape(TI, P)
        vcorr = float(NJ) * NJC * (hbc[1].sum() + hbc[3].sum())
        grand += o[0:NHALF].sum() + vcorr - o[NHALF:NBLK].sum()
    return np.float32(grand / (float(N) * float(N)))


# revision 9
# speedup vs baseline: 1.0079x; 1.0079x over previous
"""Contrastive-loss kernel for Trainium2 (8 NeuronCores, Bass/Tile).

Math (reference):
    W = wsi[:, 0, :], O = omic[:, 0, :]                      # [N, D]
    S = (W @ O.T) / max(|W_i||O_j|, eps)                     # [N, N] cosine sims
    d = diag(S)
    L = where(eye, 1 - S, relu(M - S + d[:, None]))
    out = mean(L)

Device identity:
    sum(L) = sum_{i != j} relu(hb_i - S_ij) + sum_i (1 - d_i),  hb_i = M + d_i.
    The d_i are computed exactly on the host (f64), so the device only has to
    produce sum_{i != j} relu(hb_i - S_ij). The diagonal is excluded on device
    by adding +256 to S_ii inside the PSUM accumulation group (one extra tiny
    matmul against a DoubleRow-packed scaled identity), which clamps the
    diagonal hinge to exactly 0 for both hinge formulations below.

Distribution: data-parallel over W rows. Each core c gets its 512 W rows
(pre-normalized, fp8-e4m3, DoubleRow-packed) plus the full normalized O,
column-rotated by 512*c so the diagonal block always lands in j-chunk 0
(keeps the SPMD program core-independent). Each core computes its
[512, 4096] block of S on the PE (fp8 DoubleRow, fp32 psum); the hinge and
its row-sum are fused into ONE instruction per block with accum_out,
alternating between the Scalar engine (relu(hb - S), sign +1) and the
Vector engine (min(S - hb, 0), sign -1) so neither engine becomes the
straggler. One ones-matmul collapses partitions so the output DMA is a
single 128-byte partition line.
"""

import numpy as np
import ml_dtypes

N = 4096
D = 1024
NCORES = 8
ROWS = N // NCORES  # 512 W rows per core
P = 128             # SBUF partitions
NJ = 512            # moving free dim per matmul (one PSUM bank of fp32)
TI = ROWS // P      # 4 i-tiles per core
ND2 = D // 256      # 4 DoubleRow contraction chunks (256 deep each)
NJC = N // NJ       # 8 j-chunks
MARGIN = 0.1
N_WARMUP = 7        # wide bf16 PE-warmup matmuls issued while DMAs stream
NBLK = NJC * TI     # 32 hinge blocks; even idx -> Scalar, odd idx -> Vector
NHALF = NBLK // 2   # 16 acc columns per engine
POISON = 256.0      # added to S_ii on device (16 * 16 in fp8, exact)

_cache = {}


def _build():
    from contextlib import ExitStack
    import concourse.bacc as bacc
    import concourse.tile as tile
    import concourse.mybir as mybir

    f32 = mybir.dt.float32
    bf16 = mybir.dt.bfloat16
    fp8 = mybir.dt.float8e4

    nc = bacc.Bacc("TRN2", target_bir_lowering=False, debug=False,
                   num_devices=NCORES)
    wt_d = nc.dram_tensor("wt", [P, TI * ND2, 2, P], fp8,
                          kind="ExternalInput").ap()
    ot_d = nc.dram_tensor("ot", [P, NJC * ND2, 2, NJ], fp8,
                          kind="ExternalInput").ap()
    pid_d = nc.dram_tensor("pid", [P, 2, P], fp8, kind="ExternalInput").ap()
    hb_d = nc.dram_tensor("hb", [P, TI], f32, kind="ExternalInput").ap()
    out_d = nc.dram_tensor("out", [1, NBLK], f32, kind="ExternalOutput").ap()

    with tile.TileContext(nc) as tc, ExitStack() as ctx:
        const = ctx.enter_context(tc.tile_pool(name="const", bufs=1))
        otp = ctx.enter_context(tc.tile_pool(name="otp", bufs=NJC))
        pp = ctx.enter_context(tc.tile_pool(name="pp", bufs=6, space="PSUM"))
        pp1 = ctx.enter_context(tc.tile_pool(name="pp1", bufs=1, space="PSUM"))
        scrs = ctx.enter_context(tc.tile_pool(name="scrs", bufs=2))
        scrv = ctx.enter_context(tc.tile_pool(name="scrv", bufs=2))
        smallp = ctx.enter_context(tc.tile_pool(name="small", bufs=1))

        # Input DMAs are split across BOTH hardware DGE queues (Sync + Scalar
        # engines, ~150 GB/s effective each) so the stream can start ~9us and
        # never starves. o0/o1 land earliest via partition-halves on both
        # queues; later chunks alternate whole. hb/pid ride the idle GpSimd
        # SWDGE queue.
        HP = P // 2
        wt_sb = const.tile([P, TI * ND2, 2, P], fp8, tag="wt")
        ots = [otp.tile([P, ND2, 2, NJ], fp8, tag="ot", name=f"ot{jc}")
               for jc in range(NJC)]

        def ot_half(jc, h, eng):
            sl = slice(h * HP, (h + 1) * HP)
            eng.dma_start(out=ots[jc][sl, :, :, :],
                          in_=ot_d[sl, jc * ND2:(jc + 1) * ND2, :, :])

        # sync queue: wt[t0,t1], o0a, o1a, o3, o5, o7
        nc.sync.dma_start(out=wt_sb[:, 0:2 * ND2, :, :],
                          in_=wt_d[:, 0:2 * ND2, :, :])
        ot_half(0, 0, nc.sync)
        ot_half(1, 0, nc.sync)
        for jc in (3, 5, 7):
            nc.sync.dma_start(out=ots[jc][:],
                              in_=ot_d[:, jc * ND2:(jc + 1) * ND2, :, :])
        # scalar queue: o0b, wt[t2,t3], o1b, o2, o4, o6
        ot_half(0, 1, nc.scalar)
        nc.scalar.dma_start(out=wt_sb[:, 2 * ND2:, :, :],
                            in_=wt_d[:, 2 * ND2:, :, :])
        ot_half(1, 1, nc.scalar)
        for jc in (2, 4, 6):
            nc.scalar.dma_start(out=ots[jc][:],
                                in_=ot_d[:, jc * ND2:(jc + 1) * ND2, :, :])
        # gpsimd SWDGE: the small hinge-bias + poison-identity tensors
        hb = const.tile([P, TI], f32, tag="hb")
        nc.gpsimd.dma_start(out=hb[:], in_=hb_d[:, :])
        pid_sb = const.tile([P, 2, P], fp8, tag="pid")
        nc.gpsimd.dma_start(out=pid_sb[:], in_=pid_d[:, :])
        ones_sb = const.tile([P, 1], f32, tag="ones")
        nc.vector.memset(ones_sb[:], 1.0)

        # Warm the PE clock (HAM gate releases after ~3.4us of sustained
        # array activity): wide 512-col matmuls fill the window between
        # program start and the first operand DMA landing, so the HAM
        # release timer starts ~1.5us earlier than the real stream.
        warm_w = const.tile([P, P], bf16, tag="warmw")
        nc.vector.memset(warm_w[:], 0.0)
        warm_rhs = const.tile([P, NJ], bf16, tag="warmrhs")
        nc.vector.memset(warm_rhs[:], 0.0)
        warm_ps = pp1.tile([P, NJ], f32, tag="warmps")
        for _ in range(N_WARMUP):
            nc.tensor.matmul(warm_ps[:], lhsT=warm_w[:], rhs=warm_rhs[:],
                             start=True, stop=True)

        # per-block fused hinge row-sums, one acc tile per consumer engine
        acc_s = const.tile([P, NHALF], f32, tag="accs")
        acc_v = const.tile([P, NHALF], f32, tag="accv")

        for jc in range(NJC):
            for t in range(TI):
                ps = pp.tile([P, NJ], f32, tag="ps")
                for dd in range(ND2):
                    nc.tensor.matmul(
                        ps[:],
                        lhsT=wt_sb[:, t * ND2 + dd, :, :],
                        rhs=ots[jc][:, dd, :, :],
                        start=(dd == 0),
                        stop=(dd == ND2 - 1),
                        perf_mode=mybir.MatmulPerfMode.DoubleRow,
                    )
                    if jc == 0 and dd == 0:
                        # poison the diagonal: S_ii += 256 so the hinge
                        # clamps to 0 there (diag handled analytically on
                        # host). Mid-group partial-range accumulate.
                        nc.tensor.matmul(
                            ps[:, t * P:(t + 1) * P],
                            lhsT=pid_sb[:],
                            rhs=pid_sb[:],
                            start=False,
                            stop=False,
                            perf_mode=mybir.MatmulPerfMode.DoubleRow,
                            skip_group_check=True,
                        )
                idx = jc * TI + t
                col = idx // 2
                if idx % 2 == 0:
                    # Scalar engine: acc_s[col] = sum_j relu(hb - S)
                    h = scrs.tile([P, NJ], bf16, tag="hs")
                    nc.scalar.activation(
                        out=h[:],
                        in_=ps[:],
                        func=mybir.ActivationFunctionType.Relu,
                        bias=hb[:, t:t + 1],
                        scale=-1.0,
                        accum_out=acc_s[:, col:col + 1],
                    )
                else:
                    # Vector engine: with accum_out, op1 is the REDUCTION op:
                    # acc_v[col] = sum_j min(S, hb). Host identity:
                    # sum_j relu(hb - S) = 512*hb - sum_j min(S, hb).
                    h = scrv.tile([P, NJ], bf16, tag="hv")
                    nc.vector.tensor_scalar(
                        out=h[:],
                        in0=ps[:],
                        scalar1=hb[:, t:t + 1],
                        scalar2=None,
                        op0=mybir.AluOpType.min,
                        op1=mybir.AluOpType.add,
                        accum_out=acc_v[:, col:col + 1],
                    )

        # cross-partition reduce on the PE (ones^T @ acc -> [1, 16] each) so
        # the output DMA is one contiguous partition line instead of 128
        # 4-byte descriptors (whose completion receipts dominate the tail)
        tot_ps = pp1.tile([1, NBLK], f32, tag="totps")
        nc.tensor.matmul(tot_ps[:, 0:NHALF], lhsT=ones_sb[:], rhs=acc_s[:, :],
                         start=True, stop=True)
        nc.tensor.matmul(tot_ps[:, NHALF:NBLK], lhsT=ones_sb[:],
                         rhs=acc_v[:, :], start=True, stop=True,
                         skip_group_check=True)
        total = smallp.tile([1, NBLK], f32, tag="tot")
        nc.vector.tensor_copy(total[:], tot_ps[:])
        nc.sync.dma_start(out=out_d[:, :], in_=total[:])

    nc.compile()
    return nc


def _get_nc():
    if "nc" not in _cache:
        _cache["nc"] = _build()
    return _cache["nc"]


def _prep_inputs(wsi, omic):
    fp8np = ml_dtypes.float8_e4m3
    W = np.asarray(wsi, dtype=np.float32)[:, 0, :].astype(np.float64)
    O = np.asarray(omic, dtype=np.float32)[:, 0, :].astype(np.float64)
    Wn = W / np.maximum(np.linalg.norm(W, axis=1, keepdims=True), 1e-30)
    On = O / np.maximum(np.linalg.norm(O, axis=1, keepdims=True), 1e-30)
    d_exact = np.einsum("nd,nd->n", Wn, On)  # exact cos(w_i, o_i)
    hb_all = (MARGIN + d_exact).astype(np.float32)
    Wn8 = Wn.astype(fp8np)
    On8 = On.astype(fp8np)
    # DoubleRow-packed sqrt(POISON)*I: pid[p, 0, m] = 16*delta(p, m)
    pid = np.zeros((P, 2, P), dtype=fp8np)
    pid[np.arange(P), 0, np.arange(P)] = fp8np(16.0)

    in_maps = []
    for c in range(NCORES):
        Wc = Wn8[c * ROWS:(c + 1) * ROWS]  # [512, 1024]
        # wt[p, t*ND2+dd, r, m] = Wc[t*128+m, dd*256 + r*128 + p]
        wt = np.ascontiguousarray(
            Wc.reshape(TI, P, ND2, 2, P).transpose(4, 0, 2, 3, 1)
            .reshape(P, TI * ND2, 2, P))
        # column rotation: permuted col j' <-> original O row (j' + 512c) % N
        Operm = np.roll(On8, -ROWS * c, axis=0)
        # ot[p, jc*ND2+dd, r, n] = Operm[jc*512 + n, dd*256 + r*128 + p]
        ot = np.ascontiguousarray(
            Operm.reshape(NJC, NJ, ND2, 2, P).transpose(4, 0, 2, 3, 1)
            .reshape(P, NJC * ND2, 2, NJ))
        # hb[p, t] = MARGIN + d_exact[c*512 + t*128 + p]
        hbc = np.ascontiguousarray(
            hb_all[c * ROWS:(c + 1) * ROWS].reshape(TI, P).T)
        in_maps.append({"wt": wt, "ot": ot, "pid": pid, "hb": hbc})
    return in_maps, d_exact


def kernel(wsi_embeddings, omic_embeddings):
    from concourse.bass_utils import run_bass_kernel_spmd

    nc = _get_nc()
    in_maps, d_exact = _prep_inputs(wsi_embeddings, omic_embeddings)
    res = run_bass_kernel_spmd(nc, in_maps, list(range(NCORES)))
    # device columns: out[0:16] = Scalar sums of relu(hb - S) for even
    # blocks (t in {0, 2}); out[16:32] = Vector sums of min(S, hb) for odd
    # blocks (t in {1, 3}), corrected on host via
    #   sum_j relu(hb - S) = 512*hb_i - sum_j min(S_ij, hb_i).
    # Host adds the analytic diagonal term sum_i (1 - d_i).
    hb64 = (MARGIN + d_exact).astype(np.float32).astype(np.float64)
    grand = float(np.sum(1.0 - d_exact))
    for c in range(NCORES):
        o = res.results[c]["out"].astype(np.float64).ravel()
        hbc = hb64[c * ROWS:(c + 1) * ROWS].reshape(TI, P)
        vcorr = float(NJ) * NJC * (hbc[1].sum() + hbc[3].sum())
        grand += o[0:NHALF].sum() + vcorr - o[NHALF:NBLK].sum()
    return np.float32(grand / (float(N) * float(N)))
